# revision 1
# baseline (speedup 1.0000x reference)
"""Trainium2 Bass kernel for nn_EnhancedGNN (PNA-style GNN, 4 layers).

Self-contained: host preprocessing + 8-core SPMD Bass program + runner.

 - PNA pre-MLP is linear => per-edge message = P[dst] + Q[src]; all four
   aggregations (sum / sumsq / max / min) only need per-node tables
   [Q | Q^2] (fp16, 256B rows) gathered per edge.
 - Nodes are degree-sorted and dealt round-robin to 8 cores (rank i -> core
   i%8), giving balanced per-core edge counts and identical per-tile padded
   degree K[t] across cores (required: one SPMD program).
 - Per layer: each core builds its table slice (PE matmul), AllGather of the
   fp16 table, dma_gather of CSR slot-chunks (<=1024 idx each, rotating SWDGE
   queues), DVE halving-tree segment reductions (fp16 2x mode), channel-major
   post-MLP on PE with host-folded weights (towers/scalers/lin/BN folded).
 - Pooling: per layer-stage each core transposes its own h into a node-major
   fp16 table and locally transpose-gathers it in graph order (padded with a
   -1000 dummy row; exact fp32 sum correction), then one AllReduce(add) and
   one AllReduce(max) and the head.
"""
import os
import sys

sys.path.insert(0, "/opt/trn_rl_repo")

import numpy as np

N, E, G = 20000, 320000, 32
L, T, H, IN = 4, 4, 64, 128
F = H // T
BN_EPS = 1e-5
STD_EPS = 1e-5
NC = 8
OWN = N // NC            # 2500 real nodes per core
TILES = 20
OWNP = TILES * 128       # 2560 padded
NTAB = NC * OWNP         # 20480
CHUNK = 8                # gather chunk slots (8*128 = 1024 idx <= HW cap)
POOL_W = 128             # pool slots per (core, graph)
BIGNEG = -1000.0


# ---------------------------------------------------------------- host prep

def _wrap_idx(idx_flat):
    """int16 idx stream -> SBUF wrapped layout [128, n//16]."""
    w = idx_flat.reshape(-1, 16).T          # [16, n//16]
    return np.tile(w, (8, 1)).astype(np.int16)


def preprocess(inputs):
    edge_index = np.asarray(inputs["edge_index"])
    batch = np.asarray(inputs["batch"]).astype(np.int64)
    src_o = edge_index[0].astype(np.int64)
    dst_o = edge_index[1].astype(np.int64)

    deg = np.bincount(dst_o, minlength=N).astype(np.float32)
    logd = np.log(np.maximum(deg, 1.0) + 1.0)
    avg_log = np.log(deg + 1.0).mean(dtype=np.float32)
    amp = (logd / avg_log).astype(np.float32)

    order = np.argsort(-deg, kind="stable")   # ranks: degree descending
    gid = np.empty(N, np.int64)               # old node id -> padded global id
    ranks = np.arange(N)
    gid[order] = (ranks % NC) * OWNP + ranks // NC

    src_g, dst_g = gid[src_o], gid[dst_o]
    so = np.lexsort((src_g, dst_g))
    ssrc, sdst = src_g[so], dst_g[so]
    starts = np.searchsorted(sdst, np.arange(NTAB))
    ends = np.searchsorted(sdst, np.arange(NTAB) + 1)

    K = []
    for t in range(TILES):
        kmax = 1
        for c in range(NC):
            base = c * OWNP + t * 128
            kmax = max(kmax, int((ends[base:base + 128] - starts[base:base + 128]).max()))
        K.append(kmax)

    cores = []
    for c in range(NC):
        dummy = c * OWNP + OWNP - 1
        idx_stream = []
        padk = np.zeros((128, TILES), np.float32)
        degc = np.zeros((128, TILES), np.float32)
        invdeg = np.ones((128, TILES), np.float32)
        ampc = np.ones((128, TILES), np.float32)
        invamp = np.ones((128, TILES), np.float32)
        maskc = np.zeros((128, TILES), np.float32)
        for t in range(TILES):
            k = K[t]
            tile_idx = np.empty((k, 128), np.int64)
            for p in range(128):
                n = c * OWNP + t * 128 + p
                d = int(ends[n] - starts[n])
                lst = ssrc[starts[n]:ends[n]]
                if d == 0:
                    tile_idx[:, p] = dummy
                    padk[p, t] = k
                else:
                    tile_idx[:d, p] = lst
                    tile_idx[d:, p] = lst[0]
                    padk[p, t] = k - d
                loc = t * 128 + p
                r = loc * NC + c   # global degree rank of this slot
                if loc < OWN and r < N:
                    node = order[r]
                    d0 = deg[node]
                    degc[p, t] = d0
                    invdeg[p, t] = 1.0 / max(d0, 1.0)
                    ampc[p, t] = amp[node]
                    invamp[p, t] = 1.0 / amp[node]
                    maskc[p, t] = 1.0 if d0 > 0 else 0.0
            idx_stream.append(tile_idx.astype(np.int16))
        cores.append(dict(idx=idx_stream, padk=padk, deg=degc, invdeg=invdeg,
                          amp=ampc, invamp=invamp, mask=maskc))

    # pooling: per (core, graph) own local node ids, padded to POOL_W with the
    # dummy local row (OWNP-1, whose table row is BIGNEG)
    pool_idx = np.full((NC, G, POOL_W), OWNP - 1, np.int64)
    pool_padcnt = np.zeros((NC, G), np.float32)
    for c in range(NC):
        own_nodes = order[np.arange(OWN) * NC + c]   # local i -> old node id
        b = batch[own_nodes]
        for g in range(G):
            locs = np.where(b == g)[0]
            assert len(locs) <= POOL_W, f"pool overflow {len(locs)}"
            pool_idx[c, g, :len(locs)] = locs
            pool_padcnt[c, g] = POOL_W - len(locs)

    cnt = np.bincount(batch, minlength=G).astype(np.float32)
    invcnt = np.where(cnt > 0, 1.0 / np.maximum(cnt, 1.0), 0.0).astype(np.float32)
    hasg = (cnt > 0).astype(np.float32)

    x = np.asarray(inputs["x"], np.float32)
    xT = np.zeros((NC, IN, OWNP), np.float32)
    for c in range(NC):
        xT[c, :, :OWN] = x[order[np.arange(OWN) * NC + c]].T

    return dict(cores=cores, K=K, order=order, invcnt=invcnt, hasg=hasg,
                xT=xT, pool_idx=pool_idx, pool_padcnt=pool_padcnt)


def fold_weights(inputs):
    pre_W = np.asarray(inputs["pre_W"], np.float32)
    pre_b = np.asarray(inputs["pre_b"], np.float32)
    post_W = np.asarray(inputs["post_W"], np.float32)
    post_b = np.asarray(inputs["post_b"], np.float32)
    lin_W = np.asarray(inputs["lin_W"], np.float32)
    lin_b = np.asarray(inputs["lin_b"], np.float32)
    bn_gamma = np.asarray(inputs["bn_gamma"], np.float32)
    bn_beta = np.asarray(inputs["bn_beta"], np.float32)
    bn_scale = 1.0 / np.sqrt(1.0 + BN_EPS)

    A_bd = np.zeros((L, H, H), np.float32)
    B_bd = np.zeros((L, H, H), np.float32)
    Wx = np.zeros((L, H, H), np.float32)
    W1 = np.zeros((L, 5 * H, H), np.float32)
    W2 = np.zeros((L, 5 * H, H), np.float32)
    W3 = np.zeros((L, 5 * H, H), np.float32)
    for l in range(L):
        for t in range(T):
            sl = slice(t * F, (t + 1) * F)
            A_bd[l][sl, sl] = pre_W[l, t, :F, :]
            B_bd[l][sl, sl] = pre_W[l, t, F:, :]
            Wx[l][sl, sl] = post_W[l, t, :F, :]
            for kind in range(4):
                Wm1 = post_W[l, t, F + kind * F:F + (kind + 1) * F, :]
                Wm2 = post_W[l, t, 5 * F + kind * F:5 * F + (kind + 1) * F, :]
                Wm3 = post_W[l, t, 9 * F + kind * F:9 * F + (kind + 1) * F, :]
                dstk = [1, 2, 3, 4][kind]  # CAT blocks: [P', M1, MN, MX, STD]
                for (Wm, Wt) in ((Wm1, W1), (Wm2, W2), (Wm3, W3)):
                    Wt[l][dstk * H + t * F:dstk * H + (t + 1) * F, sl] += Wm
                    if kind != 3:  # mean/mn/mx each add P'
                        Wt[l][0 * H + t * F:0 * H + (t + 1) * F, sl] += Wm
    # fold lin + BN into the z matmuls:
    # h_next = relu( (cat@W* + xt@Wx + qb) @ linW * bn_g + lin_b*bn_g + bn_b )
    Wxf = np.zeros((L, H, H), np.float32)
    W1f = np.zeros((L, 5 * H, H), np.float32)
    W2f = np.zeros((L, 5 * H, H), np.float32)
    W3f = np.zeros((L, 5 * H, H), np.float32)
    bias = np.zeros((L, H), np.float32)
    for l in range(L):
        g = bn_scale * bn_gamma[l]
        M = lin_W[l] * g[None, :]
        Wxf[l] = Wx[l] @ M
        W1f[l] = W1[l] @ M
        W2f[l] = W2[l] @ M
        W3f[l] = W3[l] @ M
        bias[l] = post_b[l].reshape(H) @ M + lin_b[l] * g + bn_beta[l]

    return dict(
        enc_W=np.asarray(inputs["enc_W"], np.float32),
        enc_b=np.asarray(inputs["enc_b"], np.float32),
        A_bd=A_bd, B_bd=B_bd, pb=pre_b.reshape(L, H).copy(),
        Wx=Wxf, W1=W1f, W2=W2f, W3=W3f, bias=bias,
        out_W1=np.asarray(inputs["out_W1"], np.float32),
        out_b1=np.asarray(inputs["out_b1"], np.float32),
        out_W2=np.asarray(inputs["out_W2"], np.float32),
        out_b2=np.asarray(inputs["out_b2"], np.float32),
    )


# ---------------------------------------------------------------- bass build

def build_program(K):
    NL = int(os.environ.get("KERNEL_LAYERS", str(L)))
    DOPOOL = bool(int(os.environ.get("KERNEL_POOL", "1")))
    import concourse.bacc as bacc
    import concourse.mybir as mybir
    import concourse.tile as tile
    from concourse.library_config import mlp
    from concourse.masks import make_identity
    from concourse.tile import add_dep_helper

    fp32 = mybir.dt.float32
    fp16 = mybir.dt.float16
    i16 = mybir.dt.int16
    AF = mybir.ActivationFunctionType
    OP = mybir.AluOpType

    KMAX = max(K)
    IDXW = sum(k * 8 for k in K)            # int16 columns of the edge idx stream

    nc = bacc.Bacc("TRN2", target_bir_lowering=False, debug=False,
                   num_devices=NC, num_swdge_queues=4)

    # ------------- I/O
    xT_d = nc.dram_tensor("xT", [IN, OWNP], fp32, kind="ExternalInput")
    eidx_d = nc.dram_tensor("eidx", [128, IDXW], i16, kind="ExternalInput")
    pidx_d = nc.dram_tensor("pidx", [128, G * POOL_W // 16], i16, kind="ExternalInput")
    consts_d = nc.dram_tensor("consts", [128, TILES, 8], fp32, kind="ExternalInput")
    gconst_d = nc.dram_tensor("gconst", [128, 3, G], fp32, kind="ExternalInput")
    encW_d = nc.dram_tensor("encW", [IN, H], fp32, kind="ExternalInput")
    encb_d = nc.dram_tensor("encb", [1, H], fp32, kind="ExternalInput")
    AB_d = nc.dram_tensor("AB", [L, 2, H, H], fp32, kind="ExternalInput")
    pb_d = nc.dram_tensor("pb", [L, 1, H], fp32, kind="ExternalInput")
    Wz_d = nc.dram_tensor("Wz", [L, 3, 3, 128, H], fp32, kind="ExternalInput")
    Wxf_d = nc.dram_tensor("Wxf", [L, H, H], fp32, kind="ExternalInput")
    bias_d = nc.dram_tensor("bias", [L, 1, H], fp32, kind="ExternalInput")
    hW1_d = nc.dram_tensor("hW1", [3, 5, H, H], fp32, kind="ExternalInput")
    hb1_d = nc.dram_tensor("hb1", [1, H], fp32, kind="ExternalInput")
    hW2_d = nc.dram_tensor("hW2", [H, 1], fp32, kind="ExternalInput")
    negrow_d = nc.dram_tensor("negrow", [1, 128], fp16, kind="ExternalInput")
    hb2_d = nc.dram_tensor("hb2", [1, 1], fp32, kind="ExternalInput")
    out_d = nc.dram_tensor("out", [G, 1], fp32, kind="ExternalOutput")

    # internal DRAM
    slice_d = nc.dram_tensor("slice_d", [OWNP, 2 * H], fp16)
    tables = [nc.dram_tensor(f"table{l}", [NTAB, 2 * H], fp16, addr_space="Shared")
              for l in range(L)]
    hnm_d = nc.dram_tensor("hnm", [OWNP, 128], fp16)
    psum_in = nc.dram_tensor("psum_in", [128, 5 * G], fp32)
    pmax_in = nc.dram_tensor("pmax_in", [128, 5 * G], fp32)
    psum_out = nc.dram_tensor("psum_out", [128, 5 * G], fp32, addr_space="Shared")
    pmax_out = nc.dram_tensor("pmax_out", [128, 5 * G], fp32, addr_space="Shared")

    with tile.TileContext(nc) as tc:
        with (
            tc.tile_pool(name="persist", bufs=1) as pers,
            tc.tile_pool(name="wpool", bufs=1) as wp,
            tc.tile_pool(name="gat", bufs=2) as gat,
            tc.tile_pool(name="tree", bufs=2) as trp,
            tc.tile_pool(name="nm", bufs=2) as nmp,
            tc.tile_pool(name="cat", bufs=10) as catp,
            tc.tile_pool(name="stage", bufs=1) as stp,
            tc.tile_pool(name="ps", bufs=3, space="PSUM") as psp,
            tc.tile_pool(name="psT", bufs=2, space="PSUM") as psT,
            tc.tile_pool(name="psZ", bufs=2, space="PSUM") as psZ,
        ):
            nc.gpsimd.load_library(mlp)

            # ---------- constants / weights
            ident = wp.tile([128, 128], fp32, tag="ident")
            make_identity(nc, ident[:])
            ident16 = wp.tile([128, 128], fp16, tag="ident16")
            nc.vector.tensor_copy(out=ident16[:], in_=ident[:])
            xT = pers.tile([IN, OWNP], fp32, tag="xT")
            nc.sync.dma_start(xT[:], xT_d[:])
            eidx = pers.tile([128, IDXW], i16, tag="eidx")
            nc.sync.dma_start(eidx[:], eidx_d[:])
            pidx = pers.tile([128, G * POOL_W // 16], i16, tag="pidx")
            nc.sync.dma_start(pidx[:], pidx_d[:])
            consts = pers.tile([128, TILES, 8], fp32, tag="consts")
            nc.sync.dma_start(consts[:], consts_d[:])
            gconst = pers.tile([128, 3, G], fp32, tag="gconst")
            nc.sync.dma_start(gconst[:], gconst_d[:])
            encW = wp.tile([IN, H], fp32, tag="encW")
            nc.sync.dma_start(encW[:], encW_d[:])
            encb = wp.tile([1, H], fp32, tag="encb")
            nc.sync.dma_start(encb[:], encb_d[:])
            ABv = wp.tile([H, L * 2 * H], fp32, tag="ABv")
            nc.sync.dma_start(ABv[:].rearrange("k (l a m) -> k l a m", l=L, a=2), AB_d.ap().rearrange("l a k m -> k l a m"))
            pbt = wp.tile([1, L * H], fp32, tag="pbt")
            nc.sync.dma_start(pbt[:].rearrange("o (l m) -> o l m", l=L), pb_d.ap().rearrange("l o m -> o l m"))
            Wz = wp.tile([128, L * 9 * H], fp32, tag="Wz")
            nc.sync.dma_start(Wz[:].rearrange("k (l w c m) -> k l w c m", l=L, w=3, c=3), Wz_d.ap().rearrange("l w c k m -> k l w c m"))
            Wxf = wp.tile([H, L * H], fp32, tag="Wxf")
            nc.sync.dma_start(Wxf[:].rearrange("k (l m) -> k l m", l=L), Wxf_d.ap().rearrange("l k m -> k l m"))
            biasT = wp.tile([1, L * H], fp32, tag="biasT")
            nc.sync.dma_start(biasT[:].rearrange("o (l m) -> o l m", l=L), bias_d.ap().rearrange("l o m -> o l m"))
            hW1 = wp.tile([H, 15 * H], fp32, tag="hW1")
            nc.sync.dma_start(hW1[:].rearrange("k (a s m) -> k a s m", a=3, s=5), hW1_d.ap().rearrange("a s k m -> k a s m"))
            hb1 = wp.tile([1, H], fp32, tag="hb1")
            nc.sync.dma_start(hb1[:], hb1_d[:])
            hW2 = wp.tile([H, 1], fp32, tag="hW2")
            nc.sync.dma_start(hW2[:], hW2_d[:])
            hb2 = wp.tile([1, 1], fp32, tag="hb2")
            nc.sync.dma_start(hb2[:], hb2_d[:])
            ones_t = wp.tile([1, OWNP], fp32, tag="ones")
            nc.vector.memset(ones_t[:], 1.0)
            eps_t = wp.tile([128, 1], fp32, tag="eps")
            nc.vector.memset(eps_t[:], STD_EPS)

            hbuf = [pers.tile([65, OWNP], fp32, tag=f"h{i}", name=f"hbuf{i}") for i in range(2)]
            for hb in hbuf:
                nc.vector.memset(hb[64:65, :], 1.0)

            pool_s = pers.tile([128, 5, G], fp32, tag="pool_s")
            pool_m = pers.tile([128, 5, G], fp32, tag="pool_m")
            if not DOPOOL:
                nc.vector.memset(pool_s[:], 0.0)
                nc.vector.memset(pool_m[:], 0.0)
            hstage = pers.tile([128, TILES, 128], fp16, tag="hstage")
            nc.gpsimd.memset(hstage[:], 0.0)

            # ---------- encoder
            h = hbuf[0]
            for cchunk in range(5):
                sl = slice(cchunk * 512, (cchunk + 1) * 512)
                ps = psp.tile([H, 512], fp32, tag="ps")
                nc.tensor.matmul(out=ps[:], lhsT=encW[:], rhs=xT[:, sl], start=True, stop=False)
                nc.tensor.matmul(out=ps[:], lhsT=encb[:], rhs=ones_t[:, sl], start=False, stop=True)
                if cchunk % 2 == 0:
                    nc.scalar.copy(out=h[0:H, sl], in_=ps[:])
                else:
                    nc.vector.tensor_copy(out=h[0:H, sl], in_=ps[:])
            nc.vector.memset(h[0:H, OWN:OWNP], 0.0)

            def pool_stage(stage, h):
                for t in range(TILES):
                    pt = psT.tile([128, H], fp32, tag="T")
                    nc.tensor.transpose(out=pt[:], in_=h[0:H, t * 128:(t + 1) * 128],
                                        identity=ident[0:H, 0:H])
                    if t % 2 == 0:
                        nc.scalar.copy(out=hstage[:, t, 0:H], in_=pt[:])
                    else:
                        nc.vector.tensor_copy(out=hstage[:, t, 0:H], in_=pt[:])
                nc.sync.dma_start(hstage[127:128, TILES - 1, :], negrow_d[:])
                wdma = nc.sync.dma_start(hnm_d.ap().rearrange("(t p) d -> p t d", p=128),
                                         hstage[:])
                for ch in range(8):
                    gp = gat.tile([128, 1, 512], fp16, tag="poolg")
                    gi = nc.gpsimd.dma_gather(
                        gp[:], hnm_d[:], pidx[:, ch * 32:(ch + 1) * 32],
                        512, 512, 128, transpose=True, queue_num=ch % 4)
                    add_dep_helper(gi.ins, wdma.ins, reason="hnm write->gather")
                    gv = gp[:].rearrange("p o (g w) -> p (o g) w", w=POOL_W)
                    nc.vector.tensor_reduce(
                        out=pool_s[:, stage, ch * 4:(ch + 1) * 4], in_=gv,
                        axis=mybir.AxisListType.X, op=OP.add)
                    nc.vector.tensor_reduce(
                        out=pool_m[:, stage, ch * 4:(ch + 1) * 4], in_=gv,
                        axis=mybir.AxisListType.X, op=OP.max)
                nc.vector.tensor_add(out=pool_s[:, stage, :], in0=pool_s[:, stage, :],
                                     in1=gconst[:, 0, :])

            if DOPOOL:
                pool_stage(0, h)

            # ---------- layers
            for l in range(NL):
                hn = hbuf[(l + 1) % 2]
                A_l = ABv[:, (l * 2 + 0) * H:(l * 2 + 1) * H]
                B_l = ABv[:, (l * 2 + 1) * H:(l * 2 + 2) * H]
                pb_l = pbt[:, l * H:(l + 1) * H]
                Wx_l = Wxf[:, l * H:(l + 1) * H]
                bias_l = biasT[:, l * H:(l + 1) * H]

                # --- table build
                tstage = stp.tile([128, TILES, 2 * H], fp16, tag="tstage")
                for t in range(TILES):
                    ps = psp.tile([128, H], fp32, tag="ps")
                    nc.tensor.matmul(out=ps[:], lhsT=h[0:H, t * 128:(t + 1) * 128],
                                     rhs=B_l, start=True, stop=True)
                    nc.vector.tensor_copy(out=tstage[:, t, 0:H], in_=ps[:])
                    nc.scalar.activation(out=tstage[:, t, H:2 * H], in_=ps[:],
                                         func=AF.Square)
                nc.sync.dma_start(slice_d.ap().rearrange("(t p) d -> p t d", p=128),
                                  tstage[:])
                cc = nc.gpsimd.collective_compute(
                    "AllGather", OP.bypass,
                    replica_groups=[list(range(NC))],
                    ins=[slice_d[:]], outs=[tables[l][:]])

                # --- per-tile aggregation + post-MLP
                qoff = 0
                qrot = 0
                for t in range(TILES):
                    k = K[t]
                    invdeg_s = consts[:, t, 1:2]
                    amp_s = consts[:, t, 2:3]
                    invamp_s = consts[:, t, 3:4]
                    mask_s = consts[:, t, 4:5]
                    padk_s = consts[:, t, 5:6]

                    gt = gat.tile([128, KMAX, 2 * H], fp16, tag="g")
                    nchunks = (k + CHUNK - 1) // CHUNK
                    for ci in range(nchunks):
                        k0 = ci * CHUNK
                        kc = min(CHUNK, k - k0)
                        gi = nc.gpsimd.dma_gather(
                            gt[:, k0:k0 + kc, :], tables[l][:],
                            eidx[:, (qoff + k0 * 8):(qoff + (k0 + kc) * 8)],
                            kc * 128, kc * 128, 2 * H, queue_num=qrot % 4)
                        qrot += 1
                        add_dep_helper(gi.ins, cc.ins, reason="allgather->gather")
                    qoff += k * 8

                    def halving(width, op, tag):
                        cur = k
                        buf = None
                        while cur > 1:
                            hh = cur // 2
                            dst = trp.tile([128, (KMAX + 1) // 2, width], fp16, tag=tag)
                            if buf is None:
                                nc.vector.tensor_tensor(
                                    out=dst[:, 0:hh, :], in0=gt[:, 0:hh, 0:width],
                                    in1=gt[:, hh:2 * hh, 0:width], op=op)
                                if cur % 2:
                                    nc.vector.tensor_copy(out=dst[:, hh, :],
                                                          in_=gt[:, cur - 1, 0:width])
                            else:
                                nc.vector.tensor_tensor(
                                    out=dst[:, 0:hh, :], in0=buf[:, 0:hh, :],
                                    in1=buf[:, hh:2 * hh, :], op=op)
                                if cur % 2:
                                    nc.vector.tensor_copy(out=dst[:, hh, :],
                                                          in_=buf[:, cur - 1, :])
                            buf = dst
                            cur = hh + cur % 2
                        return buf

                    if k > 1:
                        S_ap = halving(2 * H, OP.add, "trs")[:, 0, :]
                        MX_ap = halving(H, OP.max, "trm")[:, 0, :]
                        MN_ap = halving(H, OP.min, "trn")[:, 0, :]
                    else:
                        S_ap = gt[:, 0, :]
                        MX_ap = gt[:, 0, 0:H]
                        MN_ap = gt[:, 0, 0:H]

                    corr = nmp.tile([128, 2 * H], fp16, tag="corr")
                    nc.vector.tensor_scalar_mul(out=corr[:], in0=gt[:, 0, :], scalar1=padk_s)
                    Sc = nmp.tile([128, 2 * H], fp32, tag="Sc")
                    nc.vector.tensor_tensor(out=Sc[:], in0=S_ap, in1=corr[:],
                                            op=OP.subtract)

                    # P node-major (A matmul + pb bias row), then *mask
                    pps = psp.tile([128, H], fp32, tag="ps")
                    nc.tensor.matmul(out=pps[:], lhsT=h[0:H, t * 128:(t + 1) * 128],
                                     rhs=A_l, start=True, stop=False)
                    nc.tensor.matmul(out=pps[:], lhsT=ones_t[:, 0:128],
                                     rhs=pb_l, start=False, stop=True)
                    CATn = catp.tile([128, 3 * 128], fp16, tag="CATn")
                    CATa = catp.tile([128, 3 * 128], fp16, tag="CATa")
                    CATd = catp.tile([128, 3 * 128], fp16, tag="CATd")
                    Pp = nmp.tile([128, H], fp32, tag="Pp")
                    nc.vector.tensor_scalar_mul(out=Pp[:], in0=pps[:], scalar1=mask_s)
                    nc.vector.tensor_copy(out=CATn[:, 0:H], in_=Pp[:])
                    M1 = nmp.tile([128, H], fp32, tag="M1")
                    nc.vector.tensor_scalar_mul(out=M1[:], in0=Sc[:, 0:H], scalar1=invdeg_s)
                    nc.vector.tensor_copy(out=CATn[:, H:2 * H], in_=M1[:])
                    E2a = nmp.tile([128, H], fp32, tag="E2a")
                    nc.vector.tensor_scalar_mul(out=E2a[:], in0=Sc[:, H:2 * H],
                                                scalar1=invdeg_s)
                    nc.vector.tensor_copy(out=CATn[:, 2 * H:3 * H], in_=MN_ap)
                    nc.vector.tensor_copy(out=CATn[:, 3 * H:4 * H], in_=MX_ap)
                    mean = nmp.tile([128, H], fp32, tag="mean")
                    nc.vector.tensor_add(out=mean[:], in0=Pp[:], in1=M1[:])
                    msq = nmp.tile([128, H], fp32, tag="msq")
                    nc.scalar.activation(out=msq[:], in_=mean[:], func=AF.Square)
                    psq = nmp.tile([128, H], fp32, tag="psq")
                    nc.scalar.activation(out=psq[:], in_=Pp[:], func=AF.Square)
                    v1 = nmp.tile([128, H], fp32, tag="v1")
                    nc.vector.tensor_mul(out=v1[:], in0=Pp[:], in1=M1[:])
                    e2 = nmp.tile([128, H], fp32, tag="e2")
                    nc.vector.tensor_scalar(out=e2[:], in0=v1[:], scalar1=2.0,
                                            scalar2=None, op0=OP.mult)
                    nc.vector.tensor_add(out=e2[:], in0=e2[:], in1=psq[:])
                    nc.vector.tensor_add(out=e2[:], in0=e2[:], in1=E2a[:])
                    var = nmp.tile([128, H], fp32, tag="var")
                    nc.vector.tensor_tensor(out=var[:], in0=e2[:], in1=msq[:],
                                            op=OP.subtract)
                    nc.vector.tensor_scalar_max(out=var[:], in0=var[:], scalar1=0.0)
                    nc.scalar.activation(out=CATn[:, 4 * H:5 * H], in_=var[:],
                                         func=AF.Sqrt, bias=eps_t[:])
                    nc.vector.memset(CATn[:, 5 * H:3 * 128], 0.0)
                    nc.vector.tensor_scalar_mul(out=CATa[:], in0=CATn[:], scalar1=amp_s)
                    nc.vector.tensor_scalar_mul(out=CATd[:], in0=CATn[:], scalar1=invamp_s)

                    # transposes + z matmuls (lin/BN folded)
                    zps = psZ.tile([H, 128], fp32, tag="z")
                    nc.tensor.matmul(out=zps[:], lhsT=Wx_l,
                                     rhs=h[0:H, t * 128:(t + 1) * 128],
                                     start=True, stop=False, skip_group_check=True)
                    nc.tensor.matmul(out=zps[:], lhsT=bias_l,
                                     rhs=ones_t[:, 0:128],
                                     start=False, stop=False, skip_group_check=True)
                    for wi, CAT in enumerate((CATn, CATa, CATd)):
                        for cci in range(3):
                            tp = psT.tile([128, 128], fp16, tag="T")
                            nc.tensor.transpose(out=tp[:],
                                                in_=CAT[:, cci * 128:(cci + 1) * 128],
                                                identity=ident16[:])
                            cs = catp.tile([128, 128], fp32, tag="catS")
                            if cci % 2 == 0:
                                nc.scalar.copy(out=cs[:], in_=tp[:])
                            else:
                                nc.vector.tensor_copy(out=cs[:], in_=tp[:])
                            w_ap = Wz[:, (l * 9 + wi * 3 + cci) * H:
                                      (l * 9 + wi * 3 + cci + 1) * H]
                            last = (wi == 2 and cci == 2)
                            nc.tensor.matmul(out=zps[:], lhsT=w_ap, rhs=cs[:],
                                             start=False, stop=last,
                                             skip_group_check=True)
                    nc.vector.tensor_scalar_max(out=hn[0:H, t * 128:(t + 1) * 128],
                                                in0=zps[:], scalar1=0.0)
                nc.vector.memset(hn[0:H, OWN:OWNP], 0.0)
                h = hn
                if DOPOOL:
                    pool_stage(l + 1, h)

            # ---------- readout
            w1 = nc.sync.dma_start(psum_in[:], pool_s[:].rearrange("p s g -> p (s g)"))
            w2 = nc.sync.dma_start(pmax_in[:], pool_m[:].rearrange("p s g -> p (s g)"))
            cs1 = nc.gpsimd.collective_compute("AllReduce", OP.add,
                                               replica_groups=[list(range(NC))],
                                               ins=[psum_in[:]], outs=[psum_out[:]])
            cs2 = nc.gpsimd.collective_compute("AllReduce", OP.max,
                                               replica_groups=[list(range(NC))],
                                               ins=[pmax_in[:]], outs=[pmax_out[:]])
            gsum = pers.tile([128, 5, G], fp32, tag="gsum")
            gmax = pers.tile([128, 5, G], fp32, tag="gmax")
            r1 = nc.sync.dma_start(gsum[:], psum_out.ap().rearrange("p (s g) -> p s g", g=G))
            r2 = nc.sync.dma_start(gmax[:], pmax_out.ap().rearrange("p (s g) -> p s g", g=G))
            add_dep_helper(r1.ins, cs1.ins, reason="allreduce->read")
            add_dep_helper(r2.ins, cs2.ins, reason="allreduce->read")
            gmean = pers.tile([128, 5, G], fp32, tag="gmean")
            for s in range(5):
                nc.vector.tensor_mul(out=gmean[:, s, :], in0=gsum[:, s, :],
                                     in1=gconst[:, 1, :])
                nc.vector.tensor_mul(out=gmax[:, s, :], in0=gmax[:, s, :],
                                     in1=gconst[:, 2, :])
            ones_g = wp.tile([1, G], fp32, tag="ones_g")
            nc.vector.memset(ones_g[:], 1.0)
            zp = psp.tile([H, G], fp32, tag="ps")
            first = True
            for kind, buf in ((0, gmean), (1, gsum), (2, gmax)):
                for s in range(5):
                    nc.tensor.matmul(out=zp[:],
                                     lhsT=hW1[:, (kind * 5 + s) * H:(kind * 5 + s + 1) * H],
                                     rhs=buf[0:H, s, :], start=first, stop=False)
                    first = False
            nc.tensor.matmul(out=zp[:], lhsT=hb1[:], rhs=ones_g[:], start=False, stop=True)
            zs = pers.tile([H, G], fp32, tag="zs")
            nc.vector.tensor_scalar_max(out=zs[:], in0=zp[:], scalar1=0.0)
            op_ps = psp.tile([1, G], fp32, tag="ps")
            nc.tensor.matmul(out=op_ps[:], lhsT=hW2[:], rhs=zs[:], start=True, stop=False)
            nc.tensor.matmul(out=op_ps[:], lhsT=hb2[:], rhs=ones_g[:], start=False, stop=True)
            osb = pers.tile([1, G], fp32, tag="osb")
            nc.vector.tensor_copy(out=osb[:], in_=op_ps[:])
            nc.sync.dma_start(out_d.ap().rearrange("g o -> o g"), osb[:])

    nc.compile()
    return nc


# ---------------------------------------------------------------- runner

def kernel(**inputs):
    from concourse.bass_utils import run_bass_kernel_spmd

    pp = preprocess(inputs)
    fw = fold_weights(inputs)
    K = pp["K"]

    nc = build_program(K)

    Wz = np.zeros((L, 3, 3, 128, H), np.float32)
    for l in range(L):
        for wi, Wt in enumerate((fw["W1"], fw["W2"], fw["W3"])):
            wpad = np.zeros((384, H), np.float32)
            wpad[:5 * H] = Wt[l]
            for cci in range(3):
                Wz[l, wi, cci] = wpad[cci * 128:(cci + 1) * 128]
    AB = np.stack([np.stack([fw["A_bd"][l], fw["B_bd"][l]]) for l in range(L)])
    hW1 = np.ascontiguousarray(fw["out_W1"].reshape(3, 5, H, H))

    in_maps = []
    for c in range(NC):
        co = pp["cores"][c]
        eidx_flat = np.concatenate([co["idx"][t].reshape(-1) for t in range(TILES)])
        eidx = _wrap_idx(eidx_flat)
        pidx = _wrap_idx(pp["pool_idx"][c].reshape(-1).astype(np.int16))
        consts = np.zeros((128, TILES, 8), np.float32)
        consts[:, :, 0] = co["deg"]
        consts[:, :, 1] = co["invdeg"]
        consts[:, :, 2] = co["amp"]
        consts[:, :, 3] = co["invamp"]
        consts[:, :, 4] = co["mask"]
        consts[:, :, 5] = co["padk"]
        gconst = np.zeros((128, 3, G), np.float32)
        gconst[:, 0, :] = (-BIGNEG) * pp["pool_padcnt"][c][None, :]
        gconst[:, 1, :] = pp["invcnt"][None, :]
        gconst[:, 2, :] = pp["hasg"][None, :]
        in_maps.append({
            "xT": np.ascontiguousarray(pp["xT"][c]),
            "eidx": eidx,
            "pidx": pidx,
            "consts": consts,
            "gconst": gconst,
            "encW": fw["enc_W"],
            "encb": fw["enc_b"][None, :],
            "AB": AB,
            "pb": np.ascontiguousarray(fw["pb"][:, None, :]),
            "Wz": Wz,
            "Wxf": fw["Wx"],
            "bias": np.ascontiguousarray(fw["bias"][:, None, :]),
            "hW1": hW1,
            "hb1": fw["out_b1"][None, :],
            "hW2": fw["out_W2"],
            "hb2": fw["out_b2"].reshape(1, 1),
            "negrow": np.full((1, 128), BIGNEG, np.float16),
        })

    trace = bool(int(os.environ.get("KERNEL_TRACE", "0")))
    res = run_bass_kernel_spmd(nc, in_maps, core_ids=list(range(NC)), trace=trace)
    if trace and res.exec_time_ns is not None:
        print(f"HW exec time: {res.exec_time_ns} ns")
    out = np.asarray(res.results[0]["out"], np.float32).reshape(G, 1)
    return out



# revision 8
# speedup vs baseline: 1.4542x; 1.4542x over previous
"""Trainium2 Bass kernel for nn_EnhancedGNN (PNA-style GNN, 4 layers).

Self-contained: host preprocessing + 8-core SPMD Bass program + runner.

 - PNA pre-MLP is linear => per-edge message = P[dst] + Q[src]; all four
   aggregations (sum / sumsq / max / min) only need per-node tables
   [Q | Q^2] (fp16, 256B rows) gathered per edge.
 - Nodes are degree-sorted and dealt round-robin to 8 cores; within a core
   ranks map to (tile t, partition p).  Table token id = c*2560 + p*20 + t so
   the per-core table slice write is one contiguous DMA from staging.
 - Tiles are grouped (uniform padded degree Kg per group): ONE dma_gather per
   group per layer and halving-tree reductions batched over the whole group
   ([128, nt, k, 128] views, fp16).
 - Post-aggregation math is batched layer-wide ([128, 20, 64] tensors,
   per-tile scalars via stride-0 broadcast APs); 3 PE transposes per tile
   (PM / MXMN / SD) feed channel-major CAT stages; z-matmuls run fp16 on
   512-column groups with lin/BN/towers folded host-side.
 - Pooling: per layer-stage transposed h is staged into a [2560, 5*128]
   node-major fp16 table; ONE 5-stage transpose-gather pass at the end feeds
   the segment reduces, then AllReduce(add/max) and the head.
"""
import os
import sys

sys.path.insert(0, "/opt/trn_rl_repo")

import numpy as np

N, E, G = 20000, 320000, 32
L, T, H, IN = 4, 4, 64, 128
F = H // T
BN_EPS = 1e-5
STD_EPS = 1e-5
NC = 8
OWN = N // NC            # 2500 real nodes per core
TILES = 20
OWNP = TILES * 128       # 2560 padded
NTAB = NC * OWNP         # 20480
POOL_W = 128             # pool slots per (core, graph)
BIGNEG = -1000.0
GROUPS = [[0], [1, 2], [3, 4], [5, 6], [7, 8, 9], [10, 11, 12],
          [13, 14, 15], [16, 17, 18, 19]]


# ---------------------------------------------------------------- host prep

def _wrap_idx(idx_flat):
    """int16 idx stream -> SBUF wrapped layout [128, n//16]."""
    w = idx_flat.reshape(-1, 16).T          # [16, n//16]
    return np.tile(w, (8, 1)).astype(np.int16)


def preprocess(inputs):
    edge_index = np.asarray(inputs["edge_index"])
    batch = np.asarray(inputs["batch"]).astype(np.int64)
    src_o = edge_index[0].astype(np.int64)
    dst_o = edge_index[1].astype(np.int64)

    deg = np.bincount(dst_o, minlength=N).astype(np.float32)
    logd = np.log(np.maximum(deg, 1.0) + 1.0)
    avg_log = np.log(deg + 1.0).mean(dtype=np.float32)
    amp = (logd / avg_log).astype(np.float32)

    order = np.argsort(-deg, kind="stable")   # ranks: degree descending
    gid = np.empty(N, np.int64)               # old node id -> padded global id
    ranks = np.arange(N)
    gid[order] = (ranks % NC) * OWNP + ranks // NC

    # table token id for padded global id g=(c, t, p): c*2560 + p*20 + t
    gg = np.arange(NTAB)
    tok_of_gid = (gg // OWNP) * OWNP + (gg % 128) * TILES + (gg % OWNP) // 128

    src_g, dst_g = gid[src_o], gid[dst_o]
    so = np.lexsort((src_g, dst_g))
    ssrc, sdst = src_g[so], dst_g[so]
    starts = np.searchsorted(sdst, np.arange(NTAB))
    ends = np.searchsorted(sdst, np.arange(NTAB) + 1)

    K = []
    for t in range(TILES):
        kmax = 1
        for c in range(NC):
            base = c * OWNP + t * 128
            kmax = max(kmax, int((ends[base:base + 128] - starts[base:base + 128]).max()))
        K.append(kmax)
    KG = [max(K[t] for t in grp) for grp in GROUPS]

    cores = []
    for c in range(NC):
        dummy_tok = c * OWNP + OWNP - 1          # (p=127, t=19) pad node
        idx_stream = []
        padk = np.zeros((128, TILES), np.float32)
        degc = np.zeros((128, TILES), np.float32)
        invdeg = np.ones((128, TILES), np.float32)
        ampc = np.ones((128, TILES), np.float32)
        invamp = np.ones((128, TILES), np.float32)
        maskc = np.zeros((128, TILES), np.float32)
        for gi_, grp in enumerate(GROUPS):
            kg = KG[gi_]
            blk = np.empty((len(grp), kg, 128), np.int64)
            for ti, t in enumerate(grp):
                for p in range(128):
                    n = c * OWNP + t * 128 + p
                    d = int(ends[n] - starts[n])
                    lst = tok_of_gid[ssrc[starts[n]:ends[n]]]
                    if d == 0:
                        blk[ti, :, p] = dummy_tok
                        padk[p, t] = kg
                    else:
                        blk[ti, :d, p] = lst
                        blk[ti, d:, p] = lst[0]
                        padk[p, t] = kg - d
                    loc = t * 128 + p
                    r = loc * NC + c   # global degree rank of this slot
                    if loc < OWN and r < N:
                        node = order[r]
                        d0 = deg[node]
                        degc[p, t] = d0
                        invdeg[p, t] = 1.0 / max(d0, 1.0)
                        ampc[p, t] = amp[node]
                        invamp[p, t] = 1.0 / amp[node]
                        maskc[p, t] = 1.0 if d0 > 0 else 0.0
            idx_stream.append(blk.reshape(-1).astype(np.int16))
        cores.append(dict(idx=idx_stream, padk=padk, deg=degc, invdeg=invdeg,
                          amp=ampc, invamp=invamp, mask=maskc))

    # pooling: per (core, graph) own local node ids, padded to POOL_W with the
    # dummy local row (OWNP-1, whose hnm row is BIGNEG)
    pool_idx = np.full((NC, G, POOL_W), OWNP - 1, np.int64)
    pool_padcnt = np.zeros((NC, G), np.float32)
    for c in range(NC):
        own_nodes = order[np.arange(OWN) * NC + c]   # local i -> old node id
        b = batch[own_nodes]
        for g in range(G):
            locs = np.where(b == g)[0]
            assert len(locs) <= POOL_W, f"pool overflow {len(locs)}"
            pool_idx[c, g, :len(locs)] = locs
            pool_padcnt[c, g] = POOL_W - len(locs)

    cnt = np.bincount(batch, minlength=G).astype(np.float32)
    invcnt = np.where(cnt > 0, 1.0 / np.maximum(cnt, 1.0), 0.0).astype(np.float32)
    hasg = (cnt > 0).astype(np.float32)

    x = np.asarray(inputs["x"], np.float32)
    xT = np.zeros((NC, IN, OWNP), np.float32)
    for c in range(NC):
        xT[c, :, :OWN] = x[order[np.arange(OWN) * NC + c]].T

    return dict(cores=cores, K=K, KG=KG, order=order, invcnt=invcnt, hasg=hasg,
                xT=xT, pool_idx=pool_idx, pool_padcnt=pool_padcnt)


def fold_weights(inputs):
    pre_W = np.asarray(inputs["pre_W"], np.float32)
    pre_b = np.asarray(inputs["pre_b"], np.float32)
    post_W = np.asarray(inputs["post_W"], np.float32)
    post_b = np.asarray(inputs["post_b"], np.float32)
    lin_W = np.asarray(inputs["lin_W"], np.float32)
    lin_b = np.asarray(inputs["lin_b"], np.float32)
    bn_gamma = np.asarray(inputs["bn_gamma"], np.float32)
    bn_beta = np.asarray(inputs["bn_beta"], np.float32)
    bn_scale = 1.0 / np.sqrt(1.0 + BN_EPS)

    A_bd = np.zeros((L, H, H), np.float32)
    B_bd = np.zeros((L, H, H), np.float32)
    Wx = np.zeros((L, H, H), np.float32)
    W1 = np.zeros((L, 5 * H, H), np.float32)
    W2 = np.zeros((L, 5 * H, H), np.float32)
    W3 = np.zeros((L, 5 * H, H), np.float32)
    for l in range(L):
        for t in range(T):
            sl = slice(t * F, (t + 1) * F)
            A_bd[l][sl, sl] = pre_W[l, t, :F, :]
            B_bd[l][sl, sl] = pre_W[l, t, F:, :]
            Wx[l][sl, sl] = post_W[l, t, :F, :]
            for kind in range(4):
                Wm1 = post_W[l, t, F + kind * F:F + (kind + 1) * F, :]
                Wm2 = post_W[l, t, 5 * F + kind * F:5 * F + (kind + 1) * F, :]
                Wm3 = post_W[l, t, 9 * F + kind * F:9 * F + (kind + 1) * F, :]
                dstk = [1, 2, 3, 4][kind]  # CAT blocks: [P', M1, MN, MX, STD]
                for (Wm, Wt) in ((Wm1, W1), (Wm2, W2), (Wm3, W3)):
                    Wt[l][dstk * H + t * F:dstk * H + (t + 1) * F, sl] += Wm
                    if kind != 3:  # mean/mn/mx each add P'
                        Wt[l][0 * H + t * F:0 * H + (t + 1) * F, sl] += Wm
    # fold lin + BN into the z matmuls:
    # h_next = relu( (cat@W* + xt@Wx + qb) @ linW * bn_g + lin_b*bn_g + bn_b )
    Wxf = np.zeros((L, H, H), np.float32)
    W1f = np.zeros((L, 5 * H, H), np.float32)
    W2f = np.zeros((L, 5 * H, H), np.float32)
    W3f = np.zeros((L, 5 * H, H), np.float32)
    bias = np.zeros((L, H), np.float32)
    for l in range(L):
        g = bn_scale * bn_gamma[l]
        M = lin_W[l] * g[None, :]
        Wxf[l] = Wx[l] @ M
        W1f[l] = W1[l] @ M
        W2f[l] = W2[l] @ M
        W3f[l] = W3[l] @ M
        bias[l] = post_b[l].reshape(H) @ M + lin_b[l] * g + bn_beta[l]

    return dict(
        enc_W=np.asarray(inputs["enc_W"], np.float32),
        enc_b=np.asarray(inputs["enc_b"], np.float32),
        A_bd=A_bd, B_bd=B_bd, pb=pre_b.reshape(L, H).copy(),
        Wx=Wxf, W1=W1f, W2=W2f, W3=W3f, bias=bias,
        out_W1=np.asarray(inputs["out_W1"], np.float32),
        out_b1=np.asarray(inputs["out_b1"], np.float32),
        out_W2=np.asarray(inputs["out_W2"], np.float32),
        out_b2=np.asarray(inputs["out_b2"], np.float32),
    )


# ---------------------------------------------------------------- bass build

def build_program(KG):
    NL = int(os.environ.get("KERNEL_LAYERS", str(L)))
    DOPOOL = bool(int(os.environ.get("KERNEL_POOL", "1")))
    import concourse.bacc as bacc
    import concourse.mybir as mybir
    import concourse.tile as tile
    from concourse.library_config import mlp
    from concourse.masks import make_identity
    from concourse.tile import add_dep_helper

    fp32 = mybir.dt.float32
    fp16 = mybir.dt.float16
    i16 = mybir.dt.int16
    AF = mybir.ActivationFunctionType
    OP = mybir.AluOpType

    NGROUP = len(GROUPS)
    GTSLOTS = max(len(GROUPS[g]) * KG[g] for g in range(NGROUP))
    TRSLOTS = max(len(GROUPS[g]) * ((KG[g] + 1) // 2) for g in range(NGROUP))
    IDXW = sum(len(GROUPS[g]) * KG[g] * 8 for g in range(NGROUP))

    nc = bacc.Bacc("TRN2", target_bir_lowering=False, debug=False,
                   num_devices=NC, num_swdge_queues=4)

    # ------------- I/O
    xT_d = nc.dram_tensor("xT", [IN, OWNP], fp32, kind="ExternalInput")
    eidx_d = nc.dram_tensor("eidx", [128, IDXW], i16, kind="ExternalInput")
    pidx_d = nc.dram_tensor("pidx", [128, G * POOL_W // 16], i16, kind="ExternalInput")
    consts_d = nc.dram_tensor("consts", [128, TILES, 8], fp16, kind="ExternalInput")
    consts32_d = nc.dram_tensor("consts32", [128, TILES, 8], fp32, kind="ExternalInput")
    reps_d = nc.dram_tensor("reps", [128, 2, OWNP], fp16, kind="ExternalInput")
    gconst_d = nc.dram_tensor("gconst", [128, 3, G], fp32, kind="ExternalInput")
    encW_d = nc.dram_tensor("encW", [IN, H], fp32, kind="ExternalInput")
    encb_d = nc.dram_tensor("encb", [1, H], fp32, kind="ExternalInput")
    APB_d = nc.dram_tensor("APB", [L, 65, H], fp16, kind="ExternalInput")
    Bt_d = nc.dram_tensor("Bt", [L, H, H], fp16, kind="ExternalInput")
    Wz_d = nc.dram_tensor("Wz", [L, 3, 3, 128, H], fp16, kind="ExternalInput")
    WxB_d = nc.dram_tensor("WxB", [L, 65, H], fp16, kind="ExternalInput")
    hW1_d = nc.dram_tensor("hW1", [3, 5, H, H], fp32, kind="ExternalInput")
    hb1_d = nc.dram_tensor("hb1", [1, H], fp32, kind="ExternalInput")
    hW2_d = nc.dram_tensor("hW2", [H, 1], fp32, kind="ExternalInput")
    hb2_d = nc.dram_tensor("hb2", [1, 1], fp32, kind="ExternalInput")
    negrow_d = nc.dram_tensor("negrow", [1, 128], fp16, kind="ExternalInput")
    out_d = nc.dram_tensor("out", [G, 1], fp32, kind="ExternalOutput")

    # internal DRAM
    slice_d = nc.dram_tensor("slice_d", [128, TILES * 2 * H], fp16)
    tables = [nc.dram_tensor(f"table{l}", [NTAB, 2 * H], fp16, addr_space="Shared")
              for l in range(L)]
    hnm_d = nc.dram_tensor("hnm", [OWNP, 5 * 128], fp16)
    psum_in = nc.dram_tensor("psum_in", [128, 5 * G], fp32)
    pmax_in = nc.dram_tensor("pmax_in", [128, 5 * G], fp32)
    psum_out = nc.dram_tensor("psum_out", [128, 5 * G], fp32, addr_space="Shared")
    pmax_out = nc.dram_tensor("pmax_out", [128, 5 * G], fp32, addr_space="Shared")

    with tile.TileContext(nc) as tc:
        with (
            tc.tile_pool(name="persist", bufs=1) as pers,
            tc.tile_pool(name="wpool", bufs=1) as wp,
            tc.tile_pool(name="gat", bufs=2) as gat,
            tc.tile_pool(name="tree", bufs=2) as trp,
            tc.tile_pool(name="lay", bufs=1) as lay,
            tc.tile_pool(name="tmp", bufs=1) as tmpp,
            tc.tile_pool(name="stg", bufs=2) as stg,
            tc.tile_pool(name="catg", bufs=2) as catg,
            tc.tile_pool(name="ps", bufs=3, space="PSUM") as psp,
            tc.tile_pool(name="psT", bufs=3, space="PSUM") as psT,
            tc.tile_pool(name="psZ", bufs=2, space="PSUM") as psZ,
        ):
            nc.gpsimd.load_library(mlp)

            # ---------- constants / weights
            ident = wp.tile([128, 128], fp32, tag="ident")
            make_identity(nc, ident[:])
            ident16 = wp.tile([128, 128], fp16, tag="ident16")
            nc.vector.tensor_copy(out=ident16[:], in_=ident[:])
            xT = pers.tile([IN, OWNP], fp32, tag="xT")
            nc.sync.dma_start(xT[:], xT_d[:])
            eidx = pers.tile([128, IDXW], i16, tag="eidx")
            nc.sync.dma_start(eidx[:], eidx_d[:])
            pidx = pers.tile([128, G * POOL_W // 16], i16, tag="pidx")
            nc.sync.dma_start(pidx[:], pidx_d[:])
            consts = pers.tile([128, TILES, 8], fp16, tag="consts")
            nc.sync.dma_start(consts[:], consts_d[:])
            consts32 = pers.tile([128, TILES, 8], fp32, tag="consts32")
            nc.sync.dma_start(consts32[:], consts32_d[:])
            reps = pers.tile([128, 2, OWNP], fp16, tag="reps")
            nc.sync.dma_start(reps[:], reps_d[:])
            gconst = pers.tile([128, 3, G], fp32, tag="gconst")
            nc.sync.dma_start(gconst[:], gconst_d[:])
            encW = wp.tile([IN, H], fp32, tag="encW")
            nc.sync.dma_start(encW[:], encW_d[:])
            encb = wp.tile([1, H], fp32, tag="encb")
            nc.sync.dma_start(encb[:], encb_d[:])
            APB = wp.tile([65, L * H], fp16, tag="APB")
            nc.sync.dma_start(APB[:].rearrange("k (l m) -> k l m", l=L),
                              APB_d.ap().rearrange("l k m -> k l m"))
            Bt = wp.tile([H, L * H], fp16, tag="Bt")
            nc.sync.dma_start(Bt[:].rearrange("k (l m) -> k l m", l=L),
                              Bt_d.ap().rearrange("l k m -> k l m"))
            Wz = wp.tile([128, L * 9 * H], fp16, tag="Wz")
            nc.sync.dma_start(Wz[:].rearrange("k (l w c m) -> k l w c m", l=L, w=3, c=3),
                              Wz_d.ap().rearrange("l w c k m -> k l w c m"))
            WxB = wp.tile([65, L * H], fp16, tag="WxB")
            nc.sync.dma_start(WxB[:].rearrange("k (l m) -> k l m", l=L),
                              WxB_d.ap().rearrange("l k m -> k l m"))
            hW1 = wp.tile([H, 15 * H], fp32, tag="hW1")
            nc.sync.dma_start(hW1[:].rearrange("k (a s m) -> k a s m", a=3, s=5),
                              hW1_d.ap().rearrange("a s k m -> k a s m"))
            hb1 = wp.tile([1, H], fp32, tag="hb1")
            nc.sync.dma_start(hb1[:], hb1_d[:])
            hW2 = wp.tile([H, 1], fp32, tag="hW2")
            nc.sync.dma_start(hW2[:], hW2_d[:])
            hb2 = wp.tile([1, 1], fp32, tag="hb2")
            nc.sync.dma_start(hb2[:], hb2_d[:])
            ones_t = wp.tile([1, 512], fp32, tag="ones")
            nc.vector.memset(ones_t[:], 1.0)
            eps_t = wp.tile([128, 1], fp32, tag="eps")
            nc.vector.memset(eps_t[:], STD_EPS)

            hbuf = [pers.tile([65, OWNP], fp16, tag=f"h{i}", name=f"hbuf{i}")
                    for i in range(2)]
            for hb in hbuf:
                nc.vector.memset(hb[64:65, :], 1.0)

            pool_s = pers.tile([128, 5, G], fp32, tag="pool_s")
            pool_m = pers.tile([128, 5, G], fp32, tag="pool_m")
            if not DOPOOL:
                nc.vector.memset(pool_s[:], 0.0)
                nc.vector.memset(pool_m[:], 0.0)

            # ---------- encoder
            h = hbuf[0]
            for cchunk in range(5):
                sl = slice(cchunk * 512, (cchunk + 1) * 512)
                ps = psZ.tile([H, 512], fp32, tag="z")
                nc.tensor.matmul(out=ps[:], lhsT=encW[:], rhs=xT[:, sl], start=True, stop=False)
                nc.tensor.matmul(out=ps[:], lhsT=encb[:], rhs=ones_t[:], start=False, stop=True)
                if cchunk % 2 == 0:
                    nc.scalar.copy(out=h[0:H, sl], in_=ps[:])
                else:
                    nc.vector.tensor_copy(out=h[0:H, sl], in_=ps[:])
            nc.vector.memset(h[0:H, OWN:OWNP], 0.0)

            def pool_stage(stage, h):
                hstage = stg.tile([128, TILES, 128], fp16, tag="hstage")
                for t in range(TILES):
                    pt = psT.tile([128, 128], fp16, tag="T")
                    nc.tensor.transpose(out=pt[0:128, 0:H], in_=h[0:H, t * 128:(t + 1) * 128],
                                        identity=ident16[0:H, 0:H])
                    if t % 2 == 0:
                        nc.scalar.copy(out=hstage[:, t, 0:H], in_=pt[0:128, 0:H])
                    else:
                        nc.vector.tensor_copy(out=hstage[:, t, 0:H], in_=pt[0:128, 0:H])
                nc.vector.memset(hstage[:, :, H:128], 0.0)
                nc.sync.dma_start(hstage[127:128, TILES - 1, :], negrow_d[:])
                wdma = nc.sync.dma_start(
                    hnm_d.ap().rearrange("(t p) (s e) -> p t s e", p=128, s=5)[:, :, stage, :],
                    hstage[:])
                return wdma

            pool_wdmas = []
            if DOPOOL:
                pool_wdmas.append(pool_stage(0, h))

            # ---------- layers
            for l in range(NL):
                hn = hbuf[(l + 1) % 2]
                APB_l = APB[:, l * H:(l + 1) * H]
                Bt_l = Bt[:, l * H:(l + 1) * H]
                WxB_l = WxB[:, l * H:(l + 1) * H]

                # --- table build: [Q | Q^2] fp16 rows staged [p][t]
                tstage = lay.tile([128, TILES, 2 * H], fp16, tag="tstage")
                for t in range(TILES):
                    ps = psp.tile([128, H], fp32, tag="ps")
                    nc.tensor.matmul(out=ps[:], lhsT=h[0:H, t * 128:(t + 1) * 128],
                                     rhs=Bt_l, start=True, stop=True)
                    nc.vector.tensor_copy(out=tstage[:, t, 0:H], in_=ps[:])
                    nc.scalar.activation(out=tstage[:, t, H:2 * H], in_=ps[:],
                                         func=AF.Square)
                nc.sync.dma_start(slice_d[:], tstage[:].rearrange("p t e -> p (t e)"))
                cc = nc.gpsimd.collective_compute(
                    "AllGather", OP.bypass,
                    replica_groups=[list(range(NC))],
                    ins=[slice_d[:]], outs=[tables[l][:]])

                # --- grouped gather + batched trees
                Sagg = lay.tile([128, TILES, 2 * H], fp16, tag="Sagg")
                MM = lay.tile([128, TILES, 2 * H], fp16, tag="MM")
                qoff = 0
                qrot = 0
                for g, grp in enumerate(GROUPS):
                    nt, kg = len(grp), KG[g]
                    ts, te = grp[0], grp[-1] + 1
                    nidx = nt * kg * 128
                    gtf = gat.tile([128, GTSLOTS, 2 * H], fp16, tag="g")
                    nslots = nt * kg
                    for c0 in range(0, nslots, 8):
                        cn = min(8, nslots - c0)
                        gi = nc.gpsimd.dma_gather(
                            gtf[:, c0:c0 + cn, :], tables[l][:],
                            eidx[:, qoff + c0 * 8:qoff + (c0 + cn) * 8],
                            cn * 128, cn * 128, 2 * H, queue_num=qrot % 4)
                        qrot += 1
                        add_dep_helper(gi.ins, cc.ins, reason="allgather->gather")
                    qoff += nidx // 16
                    gt = gtf[:, 0:nt * kg, :].rearrange("p (a k) e -> p a k e", a=nt)

                    def tree(width, op, out_ap, tag):
                        cur, src, first = kg, None, True
                        while cur > 2:
                            hh = cur // 2
                            dstf = trp.tile([128, TRSLOTS, width], fp16, tag=tag)
                            dst = dstf[:, 0:nt * ((kg + 1) // 2), :].rearrange(
                                "p (a k) e -> p a k e", a=nt)
                            if first:
                                nc.vector.tensor_tensor(
                                    out=dst[:, :, 0:hh, :], in0=gt[:, :, 0:hh, 0:width],
                                    in1=gt[:, :, hh:2 * hh, 0:width], op=op)
                                if cur % 2:
                                    nc.vector.tensor_copy(out=dst[:, :, hh, :],
                                                          in_=gt[:, :, cur - 1, 0:width])
                            else:
                                nc.vector.tensor_tensor(
                                    out=dst[:, :, 0:hh, :], in0=src[:, :, 0:hh, :],
                                    in1=src[:, :, hh:2 * hh, :], op=op)
                                if cur % 2:
                                    nc.vector.tensor_copy(out=dst[:, :, hh, :],
                                                          in_=src[:, :, cur - 1, :])
                            src, first = dst, False
                            cur = hh + cur % 2
                        if first:
                            nc.vector.tensor_tensor(
                                out=out_ap, in0=gt[:, :, 0, 0:width],
                                in1=gt[:, :, 1, 0:width], op=op)
                        else:
                            nc.vector.tensor_tensor(
                                out=out_ap, in0=src[:, :, 0, :],
                                in1=src[:, :, 1, :], op=op)

                    tree(2 * H, OP.add, Sagg[:, ts:te, :], "trs")
                    tree(H, OP.max, MM[:, ts:te, H:2 * H], "trmn")
                    tree(H, OP.min, MM[:, ts:te, 0:H], "trmn")

                    # padding correction: Sc = S + gt0 * (-padk)
                    padk_bc = consts[:, ts:te, 5:6].to_broadcast([128, nt, 2 * H])
                    corr = trp.tile([128, len(GROUPS[-1]), 2 * H], fp16, tag="corr")
                    nc.vector.tensor_tensor(out=corr[:, 0:nt, :], in0=gt[:, :, 0, :],
                                            in1=padk_bc, op=OP.mult)
                    nc.vector.tensor_tensor(out=Sagg[:, ts:te, :], in0=Sagg[:, ts:te, :],
                                            in1=corr[:, 0:nt, :], op=OP.add)

                # --- P matmuls + masked Pp into PM
                PM = lay.tile([128, TILES, 2 * H], fp16, tag="PM")
                for t in range(TILES):
                    pps = psp.tile([128, H], fp32, tag="ps")
                    nc.tensor.matmul(out=pps[:], lhsT=h[:, t * 128:(t + 1) * 128],
                                     rhs=APB_l, start=True, stop=True)
                    nc.vector.tensor_scalar_mul(out=PM[:, t, 0:H], in0=pps[:],
                                                scalar1=consts32[:, t, 4:5])

                # --- batched post-math (fp32 var chain)
                invdeg_bc = consts[:, :, 1:2].to_broadcast([128, TILES, H])
                nc.vector.tensor_tensor(out=PM[:, :, H:2 * H], in0=Sagg[:, :, 0:H],
                                        in1=invdeg_bc, op=OP.mult)        # M1
                mean = tmpp.tile([128, TILES, H], fp32, tag="mean")
                nc.vector.tensor_tensor(out=mean[:], in0=PM[:, :, 0:H],
                                        in1=PM[:, :, H:2 * H], op=OP.add)
                msq = tmpp.tile([128, TILES, H], fp32, tag="msq")
                nc.scalar.activation(out=msq[:], in_=mean[:], func=AF.Square)
                tp2 = tmpp.tile([128, TILES, H], fp32, tag="tp2")
                nc.vector.tensor_tensor(out=tp2[:], in0=mean[:],
                                        in1=PM[:, :, H:2 * H], op=OP.add)  # Pp+2*M1
                e2 = tmpp.tile([128, TILES, H], fp32, tag="e2")
                nc.vector.tensor_tensor(out=e2[:], in0=PM[:, :, 0:H],
                                        in1=tp2[:], op=OP.mult)            # Pp*(Pp+2M1)
                E2a = tmpp.tile([128, TILES, H], fp32, tag="tp2")
                nc.vector.tensor_tensor(out=E2a[:], in0=Sagg[:, :, H:2 * H],
                                        in1=invdeg_bc, op=OP.mult)
                nc.vector.tensor_tensor(out=e2[:], in0=e2[:], in1=E2a[:], op=OP.add)
                var = tmpp.tile([128, TILES, H], fp32, tag="mean")
                nc.vector.tensor_tensor(out=var[:], in0=e2[:], in1=msq[:],
                                        op=OP.subtract)
                nc.vector.tensor_scalar_max(out=var[:], in0=var[:], scalar1=0.0)
                SD = lay.tile([128, TILES, 2 * H], fp16, tag="SD")
                nc.scalar.activation(out=SD[:, :, 0:H], in_=var[:],
                                     func=AF.Sqrt, bias=eps_t[:])
                nc.vector.memset(SD[:, :, H:2 * H], 0.0)

                # --- transposes -> channel-major CAT stage
                CATn = lay.tile([128, 3, OWNP], fp16, tag="CATn")
                for t in range(TILES):
                    for ci, blk in enumerate((PM, MM, SD)):
                        tp = psT.tile([128, 128], fp16, tag="T")
                        nc.tensor.transpose(out=tp[:], in_=blk[:, t, :],
                                            identity=ident16[:])
                        if (t * 3 + ci) % 2 == 0:
                            nc.scalar.copy(out=CATn[:, ci, t * 128:(t + 1) * 128],
                                           in_=tp[:])
                        else:
                            nc.vector.tensor_copy(out=CATn[:, ci, t * 128:(t + 1) * 128],
                                                  in_=tp[:])

                # --- z matmuls (fp16) per 512-col group + relu -> hn
                for zc in range(5):
                    sl = slice(zc * 512, (zc + 1) * 512)
                    CATa = catg.tile([128, 3, 512], fp16, tag="CATa")
                    CATd = catg.tile([128, 3, 512], fp16, tag="CATd")
                    nc.vector.tensor_tensor(
                        out=CATa[:], in0=CATn[:, :, sl],
                        in1=reps[:, 0:1, sl].to_broadcast([128, 3, 512]), op=OP.mult)
                    nc.vector.tensor_tensor(
                        out=CATd[:], in0=CATn[:, :, sl],
                        in1=reps[:, 1:2, sl].to_broadcast([128, 3, 512]), op=OP.mult)
                    zps = psZ.tile([H, 512], fp32, tag="z")
                    nc.tensor.matmul(out=zps[:], lhsT=WxB_l, rhs=h[:, sl],
                                     start=True, stop=False, skip_group_check=True)
                    for wi in range(3):
                        for ci in range(3):
                            w_ap = Wz[:, (l * 9 + wi * 3 + ci) * H:
                                      (l * 9 + wi * 3 + ci + 1) * H]
                            rhs = (CATn[:, ci, sl] if wi == 0 else
                                   CATa[:, ci, :] if wi == 1 else CATd[:, ci, :])
                            last = (wi == 2 and ci == 2)
                            nc.tensor.matmul(out=zps[:], lhsT=w_ap, rhs=rhs,
                                             start=False, stop=last,
                                             skip_group_check=True)
                    nc.vector.tensor_scalar_max(out=hn[0:H, sl], in0=zps[:],
                                                scalar1=0.0)
                nc.vector.memset(hn[0:H, OWN:OWNP], 0.0)
                h = hn
                if DOPOOL:
                    pool_wdmas.append(pool_stage(l + 1, h))

            # ---------- pooling: one 5-stage transpose-gather pass
            if DOPOOL:
                for ch in range(8):
                    gp = stg.tile([128, 5, 512], fp16, tag="poolg")
                    gi = nc.gpsimd.dma_gather(
                        gp[:], hnm_d[:], pidx[:, ch * 32:(ch + 1) * 32],
                        512, 512, 5 * 128, transpose=True, queue_num=ch % 4)
                    for wd in pool_wdmas:
                        add_dep_helper(gi.ins, wd.ins, reason="hnm write->gather")
                    gv = gp[:].rearrange("p s (g w) -> p (s g) w", w=POOL_W)
                    nc.vector.tensor_reduce(
                        out=pool_s[:, :, ch * 4:(ch + 1) * 4],
                        in_=gv, axis=mybir.AxisListType.X, op=OP.add)
                    nc.vector.tensor_reduce(
                        out=pool_m[:, :, ch * 4:(ch + 1) * 4],
                        in_=gv, axis=mybir.AxisListType.X, op=OP.max)
                nc.vector.tensor_tensor(
                    out=pool_s[:], in0=pool_s[:],
                    in1=gconst[:, 0:1, :].to_broadcast([128, 5, G]), op=OP.add)

            # ---------- readout
            w1 = nc.sync.dma_start(psum_in[:], pool_s[:].rearrange("p s g -> p (s g)"))
            w2 = nc.sync.dma_start(pmax_in[:], pool_m[:].rearrange("p s g -> p (s g)"))
            cs1 = nc.gpsimd.collective_compute("AllReduce", OP.add,
                                               replica_groups=[list(range(NC))],
                                               ins=[psum_in[:]], outs=[psum_out[:]])
            cs2 = nc.gpsimd.collective_compute("AllReduce", OP.max,
                                               replica_groups=[list(range(NC))],
                                               ins=[pmax_in[:]], outs=[pmax_out[:]])
            gsum = pers.tile([128, 5, G], fp32, tag="gsum")
            gmax = pers.tile([128, 5, G], fp32, tag="gmax")
            r1 = nc.sync.dma_start(gsum[:], psum_out.ap().rearrange("p (s g) -> p s g", g=G))
            r2 = nc.sync.dma_start(gmax[:], pmax_out.ap().rearrange("p (s g) -> p s g", g=G))
            add_dep_helper(r1.ins, cs1.ins, reason="allreduce->read")
            add_dep_helper(r2.ins, cs2.ins, reason="allreduce->read")
            gmean = pers.tile([128, 5, G], fp32, tag="gmean")
            nc.vector.tensor_tensor(out=gmean[:], in0=gsum[:],
                                    in1=gconst[:, 1:2, :].to_broadcast([128, 5, G]),
                                    op=OP.mult)
            nc.vector.tensor_tensor(out=gmax[:], in0=gmax[:],
                                    in1=gconst[:, 2:3, :].to_broadcast([128, 5, G]),
                                    op=OP.mult)
            ones_g = wp.tile([1, G], fp32, tag="ones_g")
            nc.vector.memset(ones_g[:], 1.0)
            zp = psp.tile([H, G], fp32, tag="ps")
            first = True
            for kind, buf in ((0, gmean), (1, gsum), (2, gmax)):
                for s in range(5):
                    nc.tensor.matmul(out=zp[:],
                                     lhsT=hW1[:, (kind * 5 + s) * H:(kind * 5 + s + 1) * H],
                                     rhs=buf[0:H, s, :], start=first, stop=False)
                    first = False
            nc.tensor.matmul(out=zp[:], lhsT=hb1[:], rhs=ones_g[:], start=False, stop=True)
            zs = pers.tile([H, G], fp32, tag="zs")
            nc.vector.tensor_scalar_max(out=zs[:], in0=zp[:], scalar1=0.0)
            op_ps = psp.tile([1, G], fp32, tag="ps")
            nc.tensor.matmul(out=op_ps[:], lhsT=hW2[:], rhs=zs[:], start=True, stop=False)
            nc.tensor.matmul(out=op_ps[:], lhsT=hb2[:], rhs=ones_g[:], start=False, stop=True)
            osb = pers.tile([1, G], fp32, tag="osb")
            nc.vector.tensor_copy(out=osb[:], in_=op_ps[:])
            nc.sync.dma_start(out_d.ap().rearrange("g o -> o g"), osb[:])

    nc.compile()
    return nc


# ---------------------------------------------------------------- runner

def kernel(**inputs):
    from concourse.bass_utils import run_bass_kernel_spmd

    pp = preprocess(inputs)
    fw = fold_weights(inputs)

    nc = build_program(pp["KG"])

    Wz = np.zeros((L, 3, 3, 128, H), np.float32)
    for l in range(L):
        for wi, Wt in enumerate((fw["W1"], fw["W2"], fw["W3"])):
            wpad = np.zeros((384, H), np.float32)
            wpad[:5 * H] = Wt[l]
            for cci in range(3):
                Wz[l, wi, cci] = wpad[cci * 128:(cci + 1) * 128]
    APB = np.zeros((L, 65, H), np.float32)
    APB[:, 0:H, :] = fw["A_bd"]
    APB[:, H, :] = fw["pb"]
    WxB = np.zeros((L, 65, H), np.float32)
    WxB[:, 0:H, :] = fw["Wx"]
    WxB[:, H, :] = fw["bias"]
    hW1 = np.ascontiguousarray(fw["out_W1"].reshape(3, 5, H, H))

    in_maps = []
    for c in range(NC):
        co = pp["cores"][c]
        eidx_flat = np.concatenate(co["idx"])
        eidx = _wrap_idx(eidx_flat)
        pidx = _wrap_idx(pp["pool_idx"][c].reshape(-1).astype(np.int16))
        consts = np.zeros((128, TILES, 8), np.float32)
        consts[:, :, 0] = co["deg"]
        consts[:, :, 1] = co["invdeg"]
        consts[:, :, 2] = co["amp"]
        consts[:, :, 3] = co["invamp"]
        consts[:, :, 4] = co["mask"]
        consts[:, :, 5] = -co["padk"]
        reps = np.zeros((128, 2, OWNP), np.float32)
        reps[:, 0, :] = co["amp"].T.reshape(-1)[None, :]
        reps[:, 1, :] = co["invamp"].T.reshape(-1)[None, :]
        gconst = np.zeros((128, 3, G), np.float32)
        gconst[:, 0, :] = (-BIGNEG) * pp["pool_padcnt"][c][None, :]
        gconst[:, 1, :] = pp["invcnt"][None, :]
        gconst[:, 2, :] = pp["hasg"][None, :]
        in_maps.append({
            "xT": np.ascontiguousarray(pp["xT"][c]),
            "eidx": eidx,
            "pidx": pidx,
            "consts": consts.astype(np.float16),
            "consts32": consts,
            "reps": reps.astype(np.float16),
            "gconst": gconst,
            "encW": fw["enc_W"],
            "encb": fw["enc_b"][None, :],
            "APB": APB.astype(np.float16),
            "Bt": fw["B_bd"].astype(np.float16),
            "Wz": Wz.astype(np.float16),
            "WxB": WxB.astype(np.float16),
            "hW1": hW1,
            "hb1": fw["out_b1"][None, :],
            "hW2": fw["out_W2"],
            "hb2": fw["out_b2"].reshape(1, 1),
            "negrow": np.full((1, 128), BIGNEG, np.float16),
        })

    trace = bool(int(os.environ.get("KERNEL_TRACE", "0")))
    res = run_bass_kernel_spmd(nc, in_maps, core_ids=list(range(NC)), trace=trace)
    if trace and res.exec_time_ns is not None:
        print(f"HW exec time: {res.exec_time_ns} ns")
    out = np.asarray(res.results[0]["out"], np.float32).reshape(G, 1)
    return out
